# revision 1
# baseline (speedup 1.0000x reference)
"""Trainium2 Bass kernel for nn_EquivariantProteinGNN (GATv2-style message passing).

Strategy (8 NeuronCores, SPMD):
  - Nodes padded to 20480 and split into 8 contiguous shards of 2560 (20 blocks
    of 128). Edges assigned to the device owning their dst node, sorted by dst,
    and packed into fixed-size per-block runs (CPB chunks of 128 edge slots,
    dummy slots excluded via one-hot masks).
  - Per layer: each device computes xl/xr for its own nodes, AllGathers xl
    (the only cross-device tensor), then processes its edge shard:
    segment-softmax (numerically safe without segment-max: logits are in
    [-1.7, 1.7]) and the message scatter are done with one-hot matmuls that
    accumulate in PSUM - no real scatter traffic.
  - Pooling: per-graph sums via one-hot matmul, per-graph maxes via masked
    per-block transposed reduces; one tiny AllGather combines partials; the
    head MLP is replicated on every device.

The program is compiled at kernel() call time; the structure constants (CPB
etc.) are derived from the actual edge data.
"""

import math
import ml_dtypes
import numpy as np

import concourse.bass as bass
import concourse.bacc as bacc
import concourse.mybir as mybir
import concourse.tile as tile
from concourse.bass_utils import run_bass_kernel_spmd
from concourse.masks import make_identity
from concourse.library_config import mlp as mlp_lib

P = 128
D = 384
H, C = 12, 32
NUM_RBF = 100
RBF_MIN, RBF_MAX = 0.0, 30.0
NEG_BIG = -1.0e30

f32 = mybir.dt.float32
bf16 = mybir.dt.bfloat16
i32 = mybir.dt.int32
i16 = mybir.dt.int16
AF = mybir.ActivationFunctionType
OP = mybir.AluOpType

# Set to False to replace Prelu with sim-supported Relu (CoreSim debugging
# only - changes semantics!)
HW_ACTS = True

# test.py hooks: set TRACE=True before calling kernel() to capture an NTFF
# profile; the raw results land in LAST_RESULTS.
TRACE = False
LAST_RESULTS = None


# --------------------------------------------------------------------------
# host-side preprocessing
# --------------------------------------------------------------------------

def prep_host(inputs, n_dev=8, G=32):
    x = np.asarray(inputs["x"], np.float32)
    pos = np.asarray(inputs["pos"], np.float32)
    edge_index = np.asarray(inputs["edge_index"], np.int64)
    batch = np.asarray(inputs["batch"], np.int64)

    N = x.shape[0]
    E = edge_index.shape[1]
    L = np.asarray(inputs["Wl"]).shape[0]

    PD = int(math.ceil(N / (n_dev * P))) * P          # nodes per device (padded)
    N_pad = PD * n_dev
    NBLK = PD // P

    src = edge_index[0].astype(np.int64)
    dst = edge_index[1].astype(np.int64)

    # edges per 128-node block
    blk = dst // P
    cnt = np.bincount(blk, minlength=N_pad // P)
    CPB = int(math.ceil(cnt.max() / P))
    EPB = CPB * P

    # slot edges: per global block, a run of EPB slots
    order = np.argsort(dst, kind="stable")
    src_s, dst_s = src[order], dst[order]
    blk_s = dst_s // P
    # position of each edge within its block run
    start = np.zeros(len(cnt), np.int64)
    start[1:] = np.cumsum(cnt)[:-1]
    within = np.arange(E) - start[blk_s]
    slot = blk_s * EPB + within                       # global slot id

    n_slots = (N_pad // P) * EPB
    g_src = np.zeros(n_slots, np.int64)
    g_dstrel = np.full(n_slots, -1.0, np.float32)
    g_psrc = np.zeros((n_slots, 3), np.float32)
    g_pdst = np.zeros((n_slots, 3), np.float32)
    g_src[slot] = src_s
    g_dstrel[slot] = (dst_s - blk_s * P).astype(np.float32)
    g_psrc[slot] = pos[src_s]
    g_pdst[slot] = pos[dst_s]

    # per-device views
    devs = []
    SPD = NBLK * EPB                                  # slots per device
    for d in range(n_dev):
        sl = slice(d * SPD, (d + 1) * SPD)
        gsr = g_src[sl].astype(np.int16).reshape(NBLK, EPB)
        gidx = np.tile(gsr.reshape(NBLK, EPB // 16, 16).transpose(0, 2, 1), (1, 8, 1)).copy()
        dr = g_dstrel[sl]
        drc = dr.reshape(NBLK, CPB, P).transpose(0, 2, 1).copy()   # [b, p, c]
        drr = dr.reshape(NBLK, EPB).astype(ml_dtypes.bfloat16)
        psrc = g_psrc[sl].reshape(NBLK, CPB, P, 3).transpose(0, 2, 1, 3).copy()  # [b, p, c, 3]
        pdst = g_pdst[sl].reshape(NBLK, CPB, P, 3).transpose(0, 2, 1, 3).copy()

        # node features, transposed for the embedding matmul
        xdev = np.zeros((PD, x.shape[1]), np.float32)
        lo, hi = d * PD, min((d + 1) * PD, N)
        if hi > lo:
            xdev[: hi - lo] = x[lo:hi]
        xT = np.ascontiguousarray(xdev.T)             # (20, PD)

        # pooling helpers
        bdev = np.full(PD, -1, np.int64)
        if hi > lo:
            bdev[: hi - lo] = batch[lo:hi]
        oh = np.zeros((PD, G), np.float32)
        real = bdev >= 0
        oh[np.arange(PD)[real], bdev[real]] = 1.0
        oh = oh.reshape(NBLK, P, G)

        devs.append(dict(gidx=gidx, drc=drc, drr=drr, psrc=psrc, pdst=pdst,
                         xT=xT, oh=oh, bdev=bdev))

    # pooling masks: per block, up to MAXG distinct graphs
    MAXG = 1
    for dv in devs:
        bdev = dv["bdev"]
        for b in range(NBLK):
            u = np.unique(bdev[b * P:(b + 1) * P])
            MAXG = max(MAXG, len(u[u >= 0]))
    for dv in devs:
        bdev = dv.pop("bdev")
        maskG = np.full((NBLK, P, MAXG), NEG_BIG, np.float32)
        cmb = np.full((G, MAXG * NBLK), NEG_BIG, np.float32)
        for b in range(NBLK):
            bb = bdev[b * P:(b + 1) * P]
            u = np.unique(bb)
            u = u[u >= 0]
            for mi, g in enumerate(u):
                maskG[b, :, mi] = np.where(bb == g, 0.0, NEG_BIG)
                cmb[g, MAXG * b + mi] = 0.0
        dv["maskAB"] = maskG
        dv["cmb"] = cmb.reshape(G, 1, MAXG * NBLK)

    # replicated parameter pack
    def bc(v):                                        # [128, n] broadcast
        v = np.asarray(v, np.float32).reshape(1, -1)
        return np.ascontiguousarray(np.broadcast_to(v, (P, v.shape[1])))

    def row(v):
        return np.asarray(v, np.float32).reshape(1, -1)

    def col(v):
        return np.asarray(v, np.float32).reshape(-1, 1)

    bn_scale = (np.asarray(inputs["bn_g"], np.float32)
                / np.sqrt(np.asarray(inputs["bn_v"], np.float32) + 1e-5))
    bn_shift = (np.asarray(inputs["bn_b"], np.float32)
                + (np.asarray(inputs["cb"], np.float32)
                   - np.asarray(inputs["bn_m"], np.float32)) * bn_scale)

    centers = np.linspace(RBF_MIN, RBF_MAX, NUM_RBF).astype(np.float32)
    spacing = (RBF_MAX - RBF_MIN) / (NUM_RBF - 1)
    gamma = 1.0 / (spacing ** 2 + 1e-8)

    att = np.asarray(inputs["att"], np.float32).reshape(L, 1, D)
    att_b = np.ascontiguousarray(np.broadcast_to(att, (L, P, D)))
    bnsc_b = np.ascontiguousarray(np.broadcast_to(bn_scale.reshape(L, 1, D), (L, P, D)))
    bnsh_b = np.ascontiguousarray(np.broadcast_to(bn_shift.reshape(L, 1, D), (L, P, D)))

    def b16(v):
        return np.asarray(v, np.float32).astype(ml_dtypes.bfloat16)

    rep = dict(
        emb_W=np.asarray(inputs["emb_W"], np.float32),
        emb_b=row(inputs["emb_b"]),
        emb_g_b=bc(inputs["emb_g"]), emb_beta_b=bc(inputs["emb_beta"]),
        eW1=b16(inputs["eW1"]),
        eb1=b16(row(inputs["eb1"])),
        eW2=b16(inputs["eW2"]),
        eb2=b16(row(inputs["eb2"])),
        e_g_col=np.asarray(inputs["e_g"], np.float32).reshape(-1, P).T.copy(),
        e_beta_col=np.asarray(inputs["e_beta"], np.float32).reshape(-1, P).T.copy(),
        Wl=b16(inputs["Wl"]), bl=b16(np.asarray(inputs["bl"]).reshape(L, 1, D)),
        Wr=b16(inputs["Wr"]), br=b16(np.asarray(inputs["br"]).reshape(L, 1, D)),
        We=b16(inputs["We"]),
        att_b=b16(att_b), bnsc_b=bnsc_b, bnsh_b=bnsh_b,
        pW=np.asarray(inputs["pW"], np.float32), pb=row(inputs["pb"]),
        hW1=np.asarray(inputs["hW1"], np.float32), hb1=row(inputs["hb1"]),
        hW2=np.asarray(inputs["hW2"], np.float32), hb2=row(inputs["hb2"]),
        hW3=np.pad(np.asarray(inputs["hW3"], np.float32), ((0, 64), (0, 0))).reshape(2, P).T.copy(),
        hb3=row(inputs["hb3"]),
        centers_b=np.ascontiguousarray(np.broadcast_to(centers.reshape(1, -1), (P, NUM_RBF))),
        iota_row=np.ascontiguousarray(np.broadcast_to(np.arange(P, dtype=np.float32), (P, P))),
        iota_col=np.arange(P, dtype=np.float32).reshape(P, 1),
    )

    meta = dict(n_dev=n_dev, N=N, E=E, G=G, L=L, PD=PD, N_pad=N_pad,
                NBLK=NBLK, CPB=CPB, EPB=EPB, gamma=gamma,
                x_in=x.shape[1], MAXG=MAXG)
    return meta, rep, devs


# --------------------------------------------------------------------------
# device program
# --------------------------------------------------------------------------

def build_program(meta):
    n_dev = meta["n_dev"]
    L, G = meta["L"], meta["G"]
    PD, N_pad = meta["PD"], meta["N_pad"]
    NBLK, CPB, EPB = meta["NBLK"], meta["CPB"], meta["EPB"]
    MAXG = meta["MAXG"]
    gamma = meta["gamma"]
    XIN = meta["x_in"]
    KD = D // P                                        # 3 feature k-chunks

    nc = bacc.Bacc(None, target_bir_lowering=False, debug=False)

    # ---- I/O ----
    def inp(name, shape, dtype=f32):
        return nc.dram_tensor(name, list(shape), dtype, kind="ExternalInput")

    gidx_d = inp("gidx", (NBLK, P, EPB // 16), i16)
    drc_d = inp("drc", (NBLK, P, CPB))
    drr_d = inp("drr", (NBLK, EPB), bf16)
    psrc_d = inp("psrc", (NBLK, P, CPB, 3))
    pdst_d = inp("pdst", (NBLK, P, CPB, 3))
    xT_d = inp("xT", (XIN, PD))
    oh_d = inp("oh", (NBLK, P, G))
    maskAB_d = inp("maskAB", (NBLK, P, MAXG))
    cmb_d = inp("cmb", (G, 1, MAXG * NBLK))

    emb_W_d = inp("emb_W", (XIN, D))
    emb_b_d = inp("emb_b", (1, D))
    emb_g_b_d = inp("emb_g_b", (P, D))
    emb_beta_b_d = inp("emb_beta_b", (P, D))
    eW1_d = inp("eW1", (NUM_RBF, D), bf16)
    eb1_d = inp("eb1", (1, D), bf16)
    eW2_d = inp("eW2", (D, D), bf16)
    eb2_d = inp("eb2", (1, D), bf16)
    e_g_col_d = inp("e_g_col", (P, KD))
    e_beta_col_d = inp("e_beta_col", (P, KD))
    Wl_d = inp("Wl", (L, D, D), bf16)
    bl_d = inp("bl", (L, 1, D), bf16)
    Wr_d = inp("Wr", (L, D, D), bf16)
    br_d = inp("br", (L, 1, D), bf16)
    We_d = inp("We", (L, D, D), bf16)
    att_b_d = inp("att_b", (L, P, D), bf16)
    bnsc_b_d = inp("bnsc_b", (L, P, D))
    bnsh_b_d = inp("bnsh_b", (L, P, D))
    pW_d = inp("pW", (2 * D, D))
    pb_d = inp("pb", (1, D))
    hW1_d = inp("hW1", (D, D))
    hb1_d = inp("hb1", (1, D))
    hW2_d = inp("hW2", (D, D // 2))
    hb2_d = inp("hb2", (1, D // 2))
    hW3_d = inp("hW3", (P, 2))
    hb3_d = inp("hb3", (1, 1))
    centers_b_d = inp("centers_b", (P, NUM_RBF))
    iota_row_d = inp("iota_row", (P, P))
    iota_col_d = inp("iota_col", (P, 1))

    out_d = nc.dram_tensor("out", [G], f32, kind="ExternalOutput")

    # internal DRAM
    encT_d = nc.dram_tensor("encT", [NBLK, KD, P, EPB], bf16)
    xl_shard_d = nc.dram_tensor("xl_shard", [PD, D], bf16)
    shared_as = "Shared" if n_dev > 4 else "Local"
    xl_full_d = nc.dram_tensor("xl_full", [N_pad, D], bf16, addr_space=shared_as)
    pool_part_d = nc.dram_tensor("pool_part", [2 * D + 1, G], f32)
    pool_all_d = nc.dram_tensor("pool_all", [n_dev * (2 * D + 1), G], f32, addr_space=shared_as)

    rg = [list(range(n_dev))]

    with tile.TileContext(nc) as tc:
        # ------- persistent pools -------
        with (
            tc.tile_pool(name="consts", bufs=1) as consts,
            tc.tile_pool(name="hpool", bufs=1) as hpool,
        ):
            nc.gpsimd.load_library(mlp_lib)
            ident = consts.tile([P, P], f32, tag="ident")
            make_identity(nc, ident)
            ident_b = consts.tile([P, P], bf16, tag="ident_b")
            make_identity(nc, ident_b)
            iota_row = consts.tile([P, P], f32, tag="iota_row")
            nc.sync.dma_start(iota_row[:], iota_row_d[:, :])
            iota_col = consts.tile([P, 1], f32, tag="iota_col")
            nc.sync.dma_start(iota_col[:], iota_col_d[:, :])
            ones_row = consts.tile([1, P], f32, tag="ones_row")
            nc.vector.memset(ones_row[:], 1.0)
            ones_col = consts.tile([P, 1], f32, tag="ones_col")
            nc.vector.memset(ones_col[:], 1.0)
            ones_row_b = consts.tile([1, P], bf16, tag="ones_row_b")
            nc.vector.memset(ones_row_b[:], 1.0)
            eps_col = consts.tile([P, 1], f32, tag="eps_col")
            nc.vector.memset(eps_col[:], 1e-5)
            eps30_col = consts.tile([P, 1], f32, tag="eps30_col")
            nc.vector.memset(eps30_col[:], 1e-30)

            silu_n = [0]

            def emit_silu(pool, out_ap, in_ap, shape):
                # silu(x) = x / (1 + exp(-x)); single-table (exp) formulation
                silu_n[0] += 1
                sn = silu_n[0]
                ex = pool.tile(shape, f32, tag="silu_ex", name=f"silu_ex{sn}")
                nc.scalar.activation(ex[:], in_ap, AF.Exp, scale=-1.0)
                nc.vector.tensor_scalar(out=ex[:], in0=ex[:], scalar1=1.0,
                                        scalar2=None, op0=OP.add)
                rcp = pool.tile(shape, f32, tag="silu_rc", name=f"silu_rc{sn}")
                nc.vector.reciprocal_approx_fast(rcp[:], ex[:])
                nc.vector.tensor_tensor(out=out_ap, in0=in_ap, in1=rcp[:], op=OP.mult)

            h_sb = [hpool.tile([P, D], f32, tag=f"h{b}", name=f"h{b}")
                    for b in range(NBLK)]

            # =========================================================
            # Stage B: node embedding  h0 = silu(LN(x @ emb_W + emb_b))
            # =========================================================
            with (
                tc.tile_pool(name="embsb", bufs=2) as embsb,
                tc.tile_pool(name="embc", bufs=1) as embc,
                tc.tile_pool(name="embps", bufs=2, space="PSUM") as embps,
            ):
                xT_sb = embc.tile([XIN, PD], f32, tag="xT")
                nc.sync.dma_start(xT_sb[:], xT_d[:, :])
                embW_sb = embc.tile([XIN, D], f32, tag="embW")
                nc.sync.dma_start(embW_sb[:], emb_W_d[:, :])
                embb_sb = embc.tile([1, D], f32, tag="embb")
                nc.sync.dma_start(embb_sb[:], emb_b_d[:, :])
                emb_g_sb = embc.tile([P, D], f32, tag="embg")
                nc.sync.dma_start(emb_g_sb[:], emb_g_b_d[:, :])
                emb_beta_sb = embc.tile([P, D], f32, tag="embbeta")
                nc.sync.dma_start(emb_beta_sb[:], emb_beta_b_d[:, :])

                for b in range(NBLK):
                    ps = embps.tile([P, D], f32, tag="ps")
                    nc.tensor.matmul(ps[:], xT_sb[:, b * P:(b + 1) * P], embW_sb[:],
                                     start=True, stop=False)
                    nc.tensor.matmul(ps[:], ones_row[:, :P], embb_sb[:],
                                     start=False, stop=True)
                    # LayerNorm over free dim
                    mu = embsb.tile([P, 1], f32, tag="mu")
                    nc.vector.tensor_reduce(out=mu[:], in_=ps[:],
                                            axis=mybir.AxisListType.X, op=OP.add)
                    nc.vector.tensor_scalar(out=mu[:], in0=mu[:], scalar1=1.0 / D,
                                            scalar2=None, op0=OP.mult)
                    xc = embsb.tile([P, D], f32, tag="xc")
                    nc.vector.tensor_scalar(out=xc[:], in0=ps[:], scalar1=mu[:, :1],
                                            scalar2=None, op0=OP.subtract)
                    sq = embsb.tile([P, D], f32, tag="sq")
                    var = embsb.tile([P, 1], f32, tag="var")
                    nc.scalar.activation(sq[:], xc[:], AF.Square, accum_out=var[:, :1])
                    lnv = embsb.tile([P, 1], f32, tag="lnv")
                    nc.scalar.activation(lnv[:], var[:], AF.Ln, scale=1.0 / D, bias=eps_col[:, :1])
                    rstd = embsb.tile([P, 1], f32, tag="rstd")
                    nc.scalar.activation(rstd[:], lnv[:], AF.Exp, scale=-0.5)
                    nc.vector.tensor_scalar(out=xc[:], in0=xc[:], scalar1=rstd[:, :1],
                                            scalar2=None, op0=OP.mult)
                    nc.vector.tensor_tensor(out=xc[:], in0=xc[:], in1=emb_g_sb[:], op=OP.mult)
                    nc.vector.tensor_tensor(out=xc[:], in0=xc[:], in1=emb_beta_sb[:], op=OP.add)
                    emit_silu(embsb, h_sb[b][:], xc[:], [P, D])

            # =========================================================
            # Stage C: edge encoder -> encT (feat-part, per block)
            # =========================================================
            with (
                tc.tile_pool(name="encsb", bufs=3) as encsb,
                tc.tile_pool(name="encw", bufs=1) as encw,
                tc.tile_pool(name="encbig", bufs=2) as encbig,
                tc.tile_pool(name="encrow", bufs=1) as encrow,
                tc.tile_pool(name="encps", bufs=3, space="PSUM") as encps,
                tc.tile_pool(name="encpr", bufs=2, space="PSUM") as encpr,
            ):
                eW1_sb = encw.tile([NUM_RBF, D], bf16, tag="eW1")
                nc.sync.dma_start(eW1_sb[:], eW1_d[:, :])
                eb1_sb = encw.tile([1, D], bf16, tag="eb1")
                nc.sync.dma_start(eb1_sb[:], eb1_d[:, :])
                eW2_sb = [encw.tile([P, D], bf16, tag=f"eW2_{k}", name=f"eW2_{k}")
                          for k in range(KD)]
                for k in range(KD):
                    nc.sync.dma_start(eW2_sb[k][:], eW2_d[k * P:(k + 1) * P, :])
                eb2_sb = encw.tile([1, D], bf16, tag="eb2")
                nc.sync.dma_start(eb2_sb[:], eb2_d[:, :])
                eg_sb = encw.tile([P, KD], f32, tag="eg")
                nc.sync.dma_start(eg_sb[:], e_g_col_d[:, :])
                ebeta_sb = encw.tile([P, KD], f32, tag="ebeta")
                nc.sync.dma_start(ebeta_sb[:], e_beta_col_d[:, :])
                centers_sb = encw.tile([P, NUM_RBF], f32, tag="centers")
                nc.sync.dma_start(centers_sb[:], centers_b_d[:, :])

                for b in range(NBLK):
                    pos_s = encbig.tile([P, CPB, 3], f32, tag="pos_s")
                    nc.sync.dma_start(pos_s[:], psrc_d[b])
                    pos_t = encbig.tile([P, CPB, 3], f32, tag="pos_t")
                    nc.sync.dma_start(pos_t[:], pdst_d[b])
                    wout = [encbig.tile([P, EPB], bf16, tag=f"wout{k}", name=f"wout{k}")
                            for k in range(KD)]
                    e2T_blk = [encbig.tile([P, EPB], f32, tag=f"e2Tb{k}", name=f"e2Tb{k}", bufs=1)
                               for k in range(KD)]
                    mu_blk = encrow.tile([1, EPB], f32, tag="mu_blk")
                    v_blk = encrow.tile([1, EPB], f32, tag="v_blk")
                    nmu_blk = encrow.tile([1, EPB], f32, tag="nmu_blk")
                    dif = encsb.tile([P, CPB, 3], f32, tag="dif")
                    nc.vector.tensor_tensor(out=dif[:], in0=pos_s[:], in1=pos_t[:],
                                            op=OP.subtract)
                    sqd = encsb.tile([P, CPB, 3], f32, tag="sqd")
                    nc.scalar.activation(sqd[:], dif[:], AF.Square)
                    d2 = encsb.tile([P, CPB], f32, tag="d2")
                    nc.vector.tensor_reduce(out=d2[:], in_=sqd[:],
                                            axis=mybir.AxisListType.X, op=OP.add)
                    lnd = encsb.tile([P, CPB], f32, tag="lnd")
                    nc.scalar.activation(lnd[:], d2[:], AF.Ln, bias=eps30_col[:, :1])
                    dist = encsb.tile([P, CPB], f32, tag="dist")
                    nc.scalar.activation(dist[:], lnd[:], AF.Exp, scale=0.5)

                    for c in range(CPB):
                        u = encsb.tile([P, NUM_RBF], f32, tag="u")
                        nc.vector.tensor_scalar(out=u[:], in0=centers_sb[:],
                                                scalar1=dist[:, c:c + 1], scalar2=None,
                                                op0=OP.subtract)
                        nc.scalar.activation(u[:], u[:], AF.Square)
                        rbf = encsb.tile([P, NUM_RBF], bf16, tag="rbf")
                        nc.scalar.activation(rbf[:], u[:], AF.Exp, scale=-float(gamma))
                        # rbfT via PE transpose
                        rbfT_ps = encps.tile([NUM_RBF, P], bf16, tag="ptb", bufs=2)
                        nc.tensor.transpose(rbfT_ps[:], rbf[:], ident_b[:])
                        rbfT = encsb.tile([NUM_RBF, P], bf16, tag="rbfT")
                        nc.vector.tensor_copy(rbfT[:], rbfT_ps[:])
                        # e1T = silu(eW1.T @ rbfT + eb1)
                        e1T = []
                        for k in range(KD):
                            pe1 = encps.tile([P, P], f32, tag="pt")
                            nc.tensor.matmul(pe1[:], eW1_sb[:, k * P:(k + 1) * P], rbfT[:],
                                             start=True, stop=False)
                            nc.tensor.matmul(pe1[:], eb1_sb[:, k * P:(k + 1) * P], ones_row_b[:],
                                             start=False, stop=True)
                            t = encsb.tile([P, P], bf16, tag=f"e1T{k}", name=f"e1T{k}")
                            emit_silu(encsb, t[:], pe1[:], [P, P])
                            e1T.append(t)
                        # e2T = eW2.T @ e1 + eb2 -> block tile
                        for m in range(KD):
                            pe2 = encps.tile([P, P], f32, tag="pt")
                            for k in range(KD):
                                nc.tensor.matmul(pe2[:], eW2_sb[k][:, m * P:(m + 1) * P],
                                                 e1T[k][:], start=(k == 0), stop=False)
                            nc.tensor.matmul(pe2[:], eb2_sb[:, m * P:(m + 1) * P], ones_row_b[:],
                                             start=False, stop=True)
                            nc.vector.tensor_copy(e2T_blk[m][:, c * P:(c + 1) * P], pe2[:])
                        # feature sums for LayerNorm via ones-matmuls
                        r1 = encpr.tile([1, P], f32, tag="pr")
                        for k in range(KD):
                            nc.tensor.matmul(r1[:], ones_col[:, :1],
                                             e2T_blk[k][:, c * P:(c + 1) * P],
                                             start=(k == 0), stop=(k == KD - 1))
                        r2 = encpr.tile([1, P], f32, tag="pr")
                        for k in range(KD):
                            sqk = encsb.tile([P, P], f32, tag="sqk")
                            nc.scalar.activation(sqk[:], e2T_blk[k][:, c * P:(c + 1) * P],
                                                 AF.Square)
                            nc.tensor.matmul(r2[:], ones_col[:, :1], sqk[:],
                                             start=(k == 0), stop=(k == KD - 1))
                        nc.vector.tensor_scalar(out=mu_blk[:, c * P:(c + 1) * P], in0=r1[:],
                                                scalar1=1.0 / D, scalar2=None, op0=OP.mult)
                        nc.vector.tensor_scalar(out=v_blk[:, c * P:(c + 1) * P], in0=r2[:],
                                                scalar1=1.0 / D, scalar2=None, op0=OP.mult)

                    # block-level LayerNorm stats (one Ln/Exp pair per block);
                    # v_blk: E[x^2] -> var -> ln -> rstd (in place); nmu doubles as scratch
                    nc.scalar.activation(nmu_blk[:], mu_blk[:], AF.Square)
                    nc.vector.tensor_tensor(out=v_blk[:], in0=v_blk[:], in1=nmu_blk[:],
                                            op=OP.subtract)
                    nc.vector.tensor_scalar(out=v_blk[:], in0=v_blk[:], scalar1=0.0,
                                            scalar2=None, op0=OP.max)
                    nc.scalar.activation(v_blk[:], v_blk[:], AF.Ln, bias=eps_col[:1, :1])
                    nc.scalar.activation(v_blk[:], v_blk[:], AF.Exp, scale=-0.5)
                    nc.vector.tensor_tensor(out=nmu_blk[:], in0=mu_blk[:], in1=v_blk[:],
                                            op=OP.mult)
                    nc.vector.tensor_scalar(out=nmu_blk[:], in0=nmu_blk[:], scalar1=-1.0,
                                            scalar2=None, op0=OP.mult)
                    # normalize pass
                    for c in range(CPB):
                        pA = encps.tile([P, P], f32, tag="pt")
                        nc.tensor.matmul(pA[:], ones_row[:, :P],
                                         v_blk[:, c * P:(c + 1) * P], start=True, stop=True)
                        pB = encps.tile([P, P], f32, tag="pt")
                        nc.tensor.matmul(pB[:], ones_row[:, :P],
                                         nmu_blk[:, c * P:(c + 1) * P], start=True, stop=True)
                        for k in range(KD):
                            t = encsb.tile([P, P], f32, tag="enrm")
                            nc.vector.tensor_tensor(out=t[:],
                                                    in0=e2T_blk[k][:, c * P:(c + 1) * P],
                                                    in1=pA[:], op=OP.mult)
                            nc.vector.tensor_tensor(out=t[:], in0=t[:], in1=pB[:], op=OP.add)
                            nc.vector.tensor_scalar(out=wout[k][:, c * P:(c + 1) * P],
                                                    in0=t[:], scalar1=eg_sb[:, k:k + 1],
                                                    scalar2=ebeta_sb[:, k:k + 1],
                                                    op0=OP.mult, op1=OP.add)
                    for k in range(KD):
                        nc.sync.dma_start(encT_d[b, k], wout[k][:])

            # =========================================================
            # Main layers
            # =========================================================
            with (
                tc.tile_pool(name="xrpool", bufs=1) as xrpool,
                tc.tile_pool(name="lw", bufs=2) as lw,
                tc.tile_pool(name="lsb", bufs=2) as lsb,
                tc.tile_pool(name="gat", bufs=2) as gat,
                tc.tile_pool(name="eetp", bufs=2) as eetp,
                tc.tile_pool(name="lps", bufs=2, space="PSUM") as lps,
                tc.tile_pool(name="lpt", bufs=2, space="PSUM") as lpt,
                tc.tile_pool(name="lpo", bufs=2, space="PSUM") as lpo,
            ):
                xr_sb = [xrpool.tile([P, D], bf16, tag=f"xr{b}", name=f"xr{b}")
                         for b in range(NBLK)]
                for layer in range(L):
                    # ---- layer weights ----
                    Wl_sb = [lw.tile([P, D], bf16, tag=f"Wl{k}", name=f"Wl{k}")
                             for k in range(KD)]
                    Wr_sb = [lw.tile([P, D], bf16, tag=f"Wr{k}", name=f"Wr{k}")
                             for k in range(KD)]
                    We_sb = [lw.tile([P, D], bf16, tag=f"We{k}", name=f"We{k}")
                             for k in range(KD)]
                    for k in range(KD):
                        nc.sync.dma_start(Wl_sb[k][:], Wl_d[layer, k * P:(k + 1) * P, :])
                        nc.sync.dma_start(Wr_sb[k][:], Wr_d[layer, k * P:(k + 1) * P, :])
                        nc.sync.dma_start(We_sb[k][:], We_d[layer, k * P:(k + 1) * P, :])
                    bl_sb = lw.tile([1, D], bf16, tag="bl")
                    nc.sync.dma_start(bl_sb[:], bl_d[layer])
                    br_sb = lw.tile([1, D], bf16, tag="br")
                    nc.sync.dma_start(br_sb[:], br_d[layer])
                    attb_sb = lw.tile([P, D], bf16, tag="attb")
                    nc.sync.dma_start(attb_sb[:], att_b_d[layer])
                    bnsc_sb = lw.tile([P, D], f32, tag="bnsc")
                    nc.sync.dma_start(bnsc_sb[:], bnsc_b_d[layer])
                    bnsh_sb = lw.tile([P, D], f32, tag="bnsh")
                    nc.sync.dma_start(bnsh_sb[:], bnsh_b_d[layer])

                    # ---- stage D: xl/xr ----
                    for b in range(NBLK):
                        hT = []
                        for k in range(KD):
                            pt = lpt.tile([P, P], f32, tag="pt")
                            nc.tensor.transpose(pt[:], h_sb[b][:, k * P:(k + 1) * P], ident[:])
                            t = lsb.tile([P, P], bf16, tag=f"hT{k}", name=f"hT{k}")
                            nc.vector.tensor_copy(t[:], pt[:])
                            hT.append(t)
                        pxl = lps.tile([P, D], f32, tag="ps")
                        for k in range(KD):
                            nc.tensor.matmul(pxl[:], hT[k][:], Wl_sb[k][:],
                                             start=(k == 0), stop=False)
                        nc.tensor.matmul(pxl[:], ones_row_b[:, :P], bl_sb[:],
                                         start=False, stop=True)
                        xl_t = lsb.tile([P, D], bf16, tag="xl_t")
                        nc.vector.tensor_copy(xl_t[:], pxl[:])
                        nc.sync.dma_start(xl_shard_d[b * P:(b + 1) * P, :], xl_t[:])
                        pxr = lps.tile([P, D], f32, tag="ps")
                        for k in range(KD):
                            nc.tensor.matmul(pxr[:], hT[k][:], Wr_sb[k][:],
                                             start=(k == 0), stop=False)
                        nc.tensor.matmul(pxr[:], ones_row_b[:, :P], br_sb[:],
                                         start=False, stop=True)
                        nc.vector.tensor_copy(xr_sb[b][:], pxr[:])

                    # ---- AllGather xl ----
                    nc.gpsimd.collective_compute(
                        "AllGather", OP.bypass, replica_groups=rg,
                        ins=[xl_shard_d[:, :]], outs=[xl_full_d[:, :]],
                    )

                    # ---- stage E: edge message passing ----
                    for b in range(NBLK):
                        drc = gat.tile([P, CPB], f32, tag="drc")
                        nc.sync.dma_start(drc[:], drc_d[b])
                        drr = gat.tile([1, EPB], bf16, tag="drr")
                        nc.sync.dma_start(drr[:], drr_d[b:b + 1, :])
                        gix = gat.tile([P, EPB // 16], i16, tag="gix")
                        nc.sync.dma_start(gix[:], gidx_d[b])
                        eet = [eetp.tile([P, EPB], bf16, tag=f"eet{k}", name=f"eet{k}")
                               for k in range(KD)]
                        for k in range(KD):
                            nc.sync.dma_start(eet[k][:], encT_d[b, k])
                        xsg = eetp.tile([P, CPB, D], bf16, tag="xsg")
                        nc.gpsimd.dma_gather(xsg[:], xl_full_d[:, :], gix[:], EPB, EPB, D,
                                             single_packet=False)
                        psum_o = lpo.tile([P, D + H], f32, tag="po")
                        for c in range(CPB):
                            xsrc = xsg[:, c]
                            prep = lpt.tile([P, P], f32, tag="pt")
                            nc.tensor.matmul(prep[:], ones_row_b[:, :P],
                                             drr[:, c * P:(c + 1) * P], start=True, stop=True)
                            ohg = lsb.tile([P, P], bf16, tag="ohg")
                            nc.vector.tensor_scalar(out=ohg[:], in0=prep[:],
                                                    scalar1=iota_col[:, :1], scalar2=None,
                                                    op0=OP.is_equal)
                            psum_s = lps.tile([P, D], f32, tag="ps")
                            for k in range(KD):
                                nc.tensor.matmul(psum_s[:], eet[k][:, c * P:(c + 1) * P],
                                                 We_sb[k][:], start=(k == 0), stop=False)
                            nc.tensor.matmul(psum_s[:], ohg[:], xr_sb[b][:],
                                             start=False, stop=True)
                            s_sb = lsb.tile([P, D], bf16, tag="s_sb")
                            nc.scalar.copy(s_sb[:], psum_s[:])
                            nc.vector.tensor_tensor(out=s_sb[:], in0=s_sb[:], in1=xsrc,
                                                    op=OP.add)
                            m_sb = lsb.tile([P, D], bf16, tag="m_sb")
                            if HW_ACTS:
                                nc.scalar.activation(m_sb[:], s_sb[:], AF.Prelu, alpha=0.2)
                            else:
                                nc.scalar.activation(m_sb[:], s_sb[:], AF.Relu)
                            t_sb = lsb.tile([P, D], bf16, tag="t_sb")
                            nc.vector.tensor_tensor(out=t_sb[:], in0=m_sb[:], in1=attb_sb[:],
                                                    op=OP.mult)
                            lg = lsb.tile([P, H], f32, tag="lg")
                            nc.vector.tensor_reduce(
                                out=lg[:], in_=t_sb[:].rearrange("p (h c) -> p h c", h=H),
                                axis=mybir.AxisListType.X, op=OP.add)
                            z_sb = lsb.tile([P, D + H], bf16, tag="z_sb")
                            nc.scalar.activation(z_sb[:, D:], lg[:], AF.Exp)
                            el_b = z_sb[:, D:].rearrange("p (h o) -> p h o", o=1).to_broadcast([P, H, C])
                            nc.vector.tensor_tensor(
                                out=z_sb[:, :D].rearrange("p (h c) -> p h c", h=H),
                                in0=xsrc.rearrange("p (h c) -> p h c", h=H),
                                in1=el_b, op=OP.mult)
                            ohs = lsb.tile([P, P], bf16, tag="ohs")
                            nc.vector.tensor_scalar(out=ohs[:], in0=iota_row[:],
                                                    scalar1=drc[:, c:c + 1], scalar2=None,
                                                    op0=OP.is_equal)
                            nc.tensor.matmul(psum_o[:], ohs[:], z_sb[:],
                                             start=(c == 0), stop=(c == CPB - 1))
                        # ---- block epilogue ----
                        den = lsb.tile([P, H], f32, tag="den")
                        nc.vector.tensor_scalar(out=den[:], in0=psum_o[:, D:],
                                                scalar1=1e-16, scalar2=None, op0=OP.add)
                        rec = lsb.tile([P, H], f32, tag="rec")
                        nc.vector.reciprocal_approx_fast(rec[:], den[:])
                        o1 = lsb.tile([P, D], f32, tag="o1")
                        rec_b = rec[:].rearrange("p (h o) -> p h o", o=1).to_broadcast([P, H, C])
                        nc.vector.tensor_tensor(
                            out=o1[:].rearrange("p (h c) -> p h c", h=H),
                            in0=psum_o[:, :D].rearrange("p (h c) -> p h c", h=H),
                            in1=rec_b, op=OP.mult)
                        nc.vector.tensor_tensor(out=o1[:], in0=o1[:], in1=bnsc_sb[:], op=OP.mult)
                        nc.vector.tensor_tensor(out=o1[:], in0=o1[:], in1=bnsh_sb[:], op=OP.add)
                        o2 = lsb.tile([P, D], f32, tag="o2")
                        emit_silu(lsb, o2[:], o1[:], [P, D])
                        nc.vector.tensor_tensor(out=h_sb[b][:], in0=h_sb[b][:], in1=o2[:],
                                                op=OP.add)

            # =========================================================
            # Stage F: pooling + head
            # =========================================================
            with (
                tc.tile_pool(name="fsb", bufs=3) as fsb,
                tc.tile_pool(name="fkeep", bufs=1) as fkeep,
                tc.tile_pool(name="fps", bufs=2, space="PSUM") as fps,
                tc.tile_pool(name="fsum", bufs=1, space="PSUM") as fsum,
            ):
                psum_sum = fsum.tile([G, D], f32, tag="psum_sum")
                psum_cnt = fsum.tile([1, G], f32, tag="psum_cnt")
                bm = [fkeep.tile([P, MAXG * NBLK], f32, tag=f"bm{k}", name=f"bm{k}")
                      for k in range(KD)]
                for b in range(NBLK):
                    ohb = fsb.tile([P, G], f32, tag="ohb")
                    nc.sync.dma_start(ohb[:], oh_d[b])
                    mab = fsb.tile([P, MAXG], f32, tag="mab")
                    nc.sync.dma_start(mab[:], maskAB_d[b])
                    nc.tensor.matmul(psum_sum[:], ohb[:], h_sb[b][:],
                                     start=(b == 0), stop=(b == NBLK - 1))
                    nc.tensor.matmul(psum_cnt[:], ones_col[:, :1], ohb[:],
                                     start=(b == 0), stop=(b == NBLK - 1))
                    for half in range(MAXG):
                        mh = fsb.tile([P, D], f32, tag="mh")
                        nc.vector.tensor_scalar(out=mh[:], in0=h_sb[b][:],
                                                scalar1=mab[:, half:half + 1], scalar2=None,
                                                op0=OP.add)
                        for k in range(KD):
                            pt = fps.tile([P, P], f32, tag="pt")
                            nc.tensor.transpose(pt[:], mh[:, k * P:(k + 1) * P], ident[:])
                            mt = fsb.tile([P, P], f32, tag="mt")
                            nc.vector.tensor_copy(mt[:], pt[:])
                            nc.vector.tensor_reduce(
                                out=bm[k][:, MAXG * b + half:MAXG * b + half + 1],
                                in_=mt[:], axis=mybir.AxisListType.X, op=OP.max)
                # combine per-graph maxes
                gmaxT = [fkeep.tile([P, G], f32, tag=f"gmaxT{k}", name=f"gmaxT{k}")
                         for k in range(KD)]
                for g in range(G):
                    cr = fsb.tile([1, MAXG * NBLK], f32, tag="cr")
                    nc.sync.dma_start(cr[:], cmb_d[g])
                    pc = fps.tile([P, MAXG * NBLK], f32, tag="pt")
                    nc.tensor.matmul(pc[:], ones_row[:, :P], cr[:], start=True, stop=True)
                    for k in range(KD):
                        mm = fsb.tile([P, MAXG * NBLK], f32, tag="mm")
                        nc.vector.tensor_tensor(out=mm[:], in0=bm[k][:], in1=pc[:], op=OP.add)
                        nc.vector.tensor_reduce(out=gmaxT[k][:, g:g + 1], in_=mm[:],
                                                axis=mybir.AxisListType.X, op=OP.max)
                # partial sums to DRAM
                sum_sb = fsb.tile([G, D], f32, tag="sum_sb")
                nc.vector.tensor_copy(sum_sb[:], psum_sum[:])
                for k in range(KD):
                    pt = fps.tile([P, G], f32, tag="pt")
                    nc.tensor.transpose(pt[:, :G], sum_sb[:, k * P:(k + 1) * P], ident[:G, :G])
                    st = fsb.tile([P, G], f32, tag="st")
                    nc.vector.tensor_copy(st[:], pt[:, :G])
                    nc.sync.dma_start(pool_part_d[k * P:(k + 1) * P, :], st[:])
                    nc.sync.dma_start(pool_part_d[D + k * P:D + (k + 1) * P, :], gmaxT[k][:])
                cntT = fsb.tile([1, G], f32, tag="cntT")
                nc.vector.tensor_copy(cntT[:], psum_cnt[:])
                nc.sync.dma_start(pool_part_d[2 * D:2 * D + 1, :], cntT[:])

                # ---- tiny AllGather of partials ----
                nc.gpsimd.collective_compute(
                    "AllGather", OP.bypass, replica_groups=rg,
                    ins=[pool_part_d[:, :]], outs=[pool_all_d[:, :]],
                )

                # ---- combine + head (replicated on all devices) ----
                n_dev_ = n_dev
                STRIDE = 2 * D + 1
                meanT = [fkeep.tile([P, G], f32, tag=f"meanT{k}", name=f"meanT{k}")
                         for k in range(KD)]
                maxT = [fkeep.tile([P, G], f32, tag=f"maxT{k}", name=f"maxT{k}")
                        for k in range(KD)]
                cnt_tot = fkeep.tile([1, G], f32, tag="cnt_tot")
                for dv in range(n_dev_):
                    base = dv * STRIDE
                    for k in range(KD):
                        ts = fsb.tile([P, G], f32, tag="ts")
                        nc.sync.dma_start(ts[:], pool_all_d[base + k * P:base + (k + 1) * P, :])
                        tm = fsb.tile([P, G], f32, tag="tm")
                        nc.sync.dma_start(tm[:], pool_all_d[base + D + k * P:base + D + (k + 1) * P, :])
                        if dv == 0:
                            nc.vector.tensor_copy(meanT[k][:], ts[:])
                            nc.vector.tensor_copy(maxT[k][:], tm[:])
                        else:
                            nc.vector.tensor_tensor(out=meanT[k][:], in0=meanT[k][:],
                                                    in1=ts[:], op=OP.add)
                            nc.vector.tensor_tensor(out=maxT[k][:], in0=maxT[k][:],
                                                    in1=tm[:], op=OP.max)
                    tc_ = fsb.tile([1, G], f32, tag="tc_")
                    nc.sync.dma_start(tc_[:], pool_all_d[base + 2 * D:base + 2 * D + 1, :])
                    if dv == 0:
                        nc.vector.tensor_copy(cnt_tot[:], tc_[:])
                    else:
                        nc.vector.tensor_tensor(out=cnt_tot[:], in0=cnt_tot[:], in1=tc_[:],
                                                op=OP.add)
                nc.vector.tensor_scalar(out=cnt_tot[:], in0=cnt_tot[:], scalar1=1.0,
                                        scalar2=None, op0=OP.max)
                inv_cnt = fkeep.tile([1, G], f32, tag="inv_cnt")
                nc.vector.reciprocal(inv_cnt[:], cnt_tot[:])
                pic = fps.tile([P, G], f32, tag="pt")
                nc.tensor.matmul(pic[:], ones_row[:, :P], inv_cnt[:], start=True, stop=True)
                for k in range(KD):
                    nc.vector.tensor_tensor(out=meanT[k][:], in0=meanT[k][:], in1=pic[:],
                                            op=OP.mult)
                hgT = meanT + maxT          # 6 k-tiles of [128, G] = hg transposed

                # head weights
                pW_sb = [fkeep.tile([P, D], f32, tag=f"pW{k}", name=f"pW{k}")
                         for k in range(2 * KD)]
                for k in range(2 * KD):
                    nc.sync.dma_start(pW_sb[k][:], pW_d[k * P:(k + 1) * P, :])
                pb_sb = fkeep.tile([1, D], f32, tag="pb")
                nc.sync.dma_start(pb_sb[:], pb_d[:, :])
                hW1_sb = [fkeep.tile([P, D], f32, tag=f"hW1_{k}", name=f"hW1_{k}")
                          for k in range(KD)]
                for k in range(KD):
                    nc.sync.dma_start(hW1_sb[k][:], hW1_d[k * P:(k + 1) * P, :])
                hb1_sb = fkeep.tile([1, D], f32, tag="hb1")
                nc.sync.dma_start(hb1_sb[:], hb1_d[:, :])
                hW2_sb = [fkeep.tile([P, D // 2], f32, tag=f"hW2_{k}", name=f"hW2_{k}")
                          for k in range(KD)]
                for k in range(KD):
                    nc.sync.dma_start(hW2_sb[k][:], hW2_d[k * P:(k + 1) * P, :])
                hb2_sb = fkeep.tile([1, D // 2], f32, tag="hb2")
                nc.sync.dma_start(hb2_sb[:], hb2_d[:, :])
                hW3_sb = fkeep.tile([P, 2], f32, tag="hW3")
                nc.sync.dma_start(hW3_sb[:], hW3_d[:, :].rearrange("(k p) o -> p (k o)", p=P))
                hb3_sb = fkeep.tile([1, 1], f32, tag="hb3")
                nc.sync.dma_start(hb3_sb[:], hb3_d[:, :])

                def mlp_layer(in_tiles, W_tiles, b_row, out_feats, lid, act=True):
                    outs = []
                    n_out_tiles = (out_feats + P - 1) // P
                    for m in range(n_out_tiles):
                        mw = min(P, out_feats - m * P)
                        pm = fps.tile([P, G], f32, tag="ph", name=f"ph{lid}_{m}")
                        for k, (it, wt) in enumerate(zip(in_tiles, W_tiles)):
                            nc.tensor.matmul(pm[:mw, :], wt[:, m * P:m * P + mw], it[:],
                                             start=(k == 0), stop=False)
                        nc.tensor.matmul(pm[:mw, :], b_row[:, m * P:m * P + mw],
                                         ones_row[:, :G], start=False, stop=True)
                        ot = fkeep.tile([P, G], f32, tag=f"ot{lid}_{m}", name=f"ot{lid}_{m}")
                        if mw < P:
                            nc.vector.memset(ot[mw:, :], 0.0)
                        if act:
                            emit_silu(fsb, ot[:mw, :], pm[:mw, :], [mw, G])
                        else:
                            nc.vector.tensor_copy(ot[:mw, :], pm[:mw, :])
                        outs.append(ot)
                    return outs

                h1 = mlp_layer(hgT, pW_sb, pb_sb, D, 1)
                h2 = mlp_layer(h1, hW1_sb, hb1_sb, D, 2)
                h3 = mlp_layer(h2, hW2_sb, hb2_sb, D // 2, 3)
                # final: out[1, G] = hW3.T @ h3 + hb3  (contraction over 192)
                pf = fps.tile([1, G], f32, tag="pf")
                nc.tensor.matmul(pf[:], hW3_sb[:, 0:1], h3[0][:], start=True, stop=False)
                nc.tensor.matmul(pf[:], hW3_sb[:, 1:2], h3[1][:], start=False, stop=False)
                nc.tensor.matmul(pf[:], hb3_sb[:, :1], ones_row[:, :G], start=False, stop=True)
                fo = fsb.tile([1, G], f32, tag="fo")
                nc.vector.tensor_copy(fo[:], pf[:])
                nc.sync.dma_start(out_d[:].rearrange("(o g) -> o g", o=1), fo[:])

    nc.compile()
    return nc


# --------------------------------------------------------------------------
# entry point
# --------------------------------------------------------------------------

def kernel(**inputs):
    n_dev = 8
    meta, rep, devs = prep_host(inputs, n_dev)
    nc = build_program(meta)

    in_maps = []
    for d in range(n_dev):
        m = dict(rep)
        m.update(devs[d])
        in_maps.append(m)

    global LAST_RESULTS
    res = run_bass_kernel_spmd(nc, in_maps, core_ids=list(range(n_dev)),
                               trace=TRACE)
    LAST_RESULTS = res
    out = np.asarray(res.results[0]["out"], np.float32)
    return out



# revision 19
# speedup vs baseline: 1.0577x; 1.0577x over previous
"""Trainium2 Bass kernel for nn_EquivariantProteinGNN (GATv2-style message passing).

v2 — restructured from the 7.7ms baseline around the measured bottlenecks:
  - Edge encoder (stage C) rebuilt wide + feature-major: rank-1 log-RBF via
    K=1 outer-product matmuls + one Exp; silu/LN stats via wide ops; the
    LayerNorm (-mu)*rstd is folded into the stored edge features and
    g/beta are folded into We/bl on the host, so layers consume e2s directly.
  - Per-edge one-hot gather/scatter matrices precomputed on host in fp8 and
    streamed from DRAM (no per-chunk DVE is_equal / prep matmuls).
  - xl[src] added into the logit PSUM via an identity matmul; leaky-relu runs
    on ACT (Prelu) straight out of PSUM; all remaining elementwise work is
    block-wide (CPB*384 at a time) split across DVE / Pool(gpsimd) / ACT.
  - AllGather of xl split in two halves overlapped with the second half of
    stage D.
  - All ACT functions kept inside the natural_log_exp table set (exp, ln,
    square, prelu, copy) - zero table thrashing.

Sharding: nodes 0..19999 padded to 20480, 2560 per device (20 blocks of 128);
edges assigned to the device owning their dst node; ~4M params replicated.
"""

import math
import ml_dtypes
import numpy as np

import concourse.bass as bass
import concourse.bacc as bacc
import concourse.mybir as mybir
import concourse.tile as tile
from concourse.bass_utils import run_bass_kernel_spmd
from concourse.masks import make_identity
from concourse.library_config import mlp as mlp_lib

P = 128
D = 384
H, C = 12, 32
KD = D // P
NUM_RBF = 100
RBF_MIN, RBF_MAX = 0.0, 30.0
NEG_BIG = -1.0e30

f32 = mybir.dt.float32
bf16 = mybir.dt.bfloat16
fp8 = mybir.dt.float8e4
i32 = mybir.dt.int32
i16 = mybir.dt.int16
AF = mybir.ActivationFunctionType
OP = mybir.AluOpType

np_bf16 = ml_dtypes.bfloat16
np_fp8 = ml_dtypes.float8_e4m3

TRACE = False
LAST_RESULTS = None


# --------------------------------------------------------------------------
# host-side preprocessing
# --------------------------------------------------------------------------

def prep_host(inputs, n_dev=8, G=32):
    x = np.asarray(inputs["x"], np.float32)
    pos = np.asarray(inputs["pos"], np.float32)
    edge_index = np.asarray(inputs["edge_index"], np.int64)
    batch = np.asarray(inputs["batch"], np.int64)

    N = x.shape[0]
    E = edge_index.shape[1]
    L = np.asarray(inputs["Wl"]).shape[0]

    PD = int(math.ceil(N / (n_dev * P))) * P          # nodes per device (padded)
    N_pad = PD * n_dev
    NBLK = PD // P
    HB = PD // 2                                      # half-shard rows (AG split)

    src = edge_index[0].astype(np.int64)
    dst = edge_index[1].astype(np.int64)

    blk = dst // P
    cnt = np.bincount(blk, minlength=N_pad // P)
    CPB = int(math.ceil(cnt.max() / P))
    EPB = CPB * P

    order = np.argsort(dst, kind="stable")
    src_s, dst_s = src[order], dst[order]
    blk_s = dst_s // P
    start = np.zeros(len(cnt), np.int64)
    start[1:] = np.cumsum(cnt)[:-1]
    within = np.arange(E) - start[blk_s]
    slot = blk_s * EPB + within                       # global slot id

    n_slots = (N_pad // P) * EPB
    g_src = np.zeros(n_slots, np.int64)               # padding -> node 0 (harmless)
    g_dstrel = np.full(n_slots, -1, np.int64)         # padding -> -1 (no one-hot)
    g_dist = np.zeros(n_slots, np.float32)
    g_src[slot] = src_s
    g_dstrel[slot] = dst_s - blk_s * P
    g_dist[slot] = np.linalg.norm(pos[src_s] - pos[dst_s], axis=-1)

    # xl_full row remap for the split AllGather layout:
    # row(g) = half*(n_dev*HB) + dev*HB + off%HB, half = (g%PD)//HB
    def remap(g):
        dev = g // PD
        off = g % PD
        half = off // HB
        return half * (n_dev * HB) + dev * HB + (off % HB)

    g_src_r = remap(g_src)

    spacing = (RBF_MAX - RBF_MIN) / (NUM_RBF - 1)
    gamma = 1.0 / (spacing ** 2 + 1e-8)
    centers = np.linspace(RBF_MIN, RBF_MAX, NUM_RBF).astype(np.float64)

    devs = []
    SPD = NBLK * EPB
    for d in range(n_dev):
        sl = slice(d * SPD, (d + 1) * SPD)
        gsr = g_src_r[sl].astype(np.int16).reshape(NBLK, EPB)
        gidx = np.tile(gsr.reshape(NBLK, EPB // 16, 16).transpose(0, 2, 1), (1, 8, 1)).copy()
        dr = g_dstrel[sl].reshape(NBLK, EPB)
        dd = g_dist[sl].reshape(NBLK, EPB).astype(np.float64)

        # rbf rows: [NBLK, 2, EPB]: row0 = dist, row1 = -gamma*dist^2
        rbf_rows = np.stack([dd, -gamma * dd * dd], axis=1).astype(np.float32)

        # one-hot gather (ohg: [node, slot]) / scatter (ohs: [slot, node]) in fp8
        ohg = np.zeros((NBLK, P, CPB, P), np_fp8)
        ohs = np.zeros((NBLK, P, CPB, P), np_fp8)
        for b in range(NBLK):
            for c in range(CPB):
                rel = dr[b, c * P:(c + 1) * P]        # dst-rel of slots in chunk c
                valid = rel >= 0
                ss = np.arange(P)[valid]
                rr = rel[valid]
                ohg[b, rr, c, ss] = 1.0
                ohs[b, ss, c, rr] = 1.0

        xdev = np.zeros((PD, x.shape[1]), np.float32)
        lo, hi = d * PD, min((d + 1) * PD, N)
        if hi > lo:
            xdev[: hi - lo] = x[lo:hi]
        xT = np.ascontiguousarray(xdev.T)

        bdev = np.full(PD, -1, np.int64)
        if hi > lo:
            bdev[: hi - lo] = batch[lo:hi]
        oh = np.zeros((PD, G), np.float32)
        real = bdev >= 0
        oh[np.arange(PD)[real], bdev[real]] = 1.0
        oh = oh.reshape(NBLK, P, G)

        devs.append(dict(gidx=gidx, rbf_rows=rbf_rows, ohg=ohg, ohs=ohs,
                         xT=xT, oh=oh, bdev=bdev))

    # pooling masks
    MAXG = 1
    for dv in devs:
        bdev = dv["bdev"]
        for b in range(NBLK):
            u = np.unique(bdev[b * P:(b + 1) * P])
            MAXG = max(MAXG, len(u[u >= 0]))
    for dv in devs:
        bdev = dv.pop("bdev")
        maskG = np.full((NBLK, P, MAXG), NEG_BIG, np.float32)
        cmb = np.full((G, MAXG * NBLK), NEG_BIG, np.float32)
        for b in range(NBLK):
            bb = bdev[b * P:(b + 1) * P]
            u = np.unique(bb)
            u = u[u >= 0]
            for mi, g in enumerate(u):
                maskG[b, :, mi] = np.where(bb == g, 0.0, NEG_BIG)
                cmb[g, MAXG * b + mi] = 0.0
        dv["maskAB"] = maskG
        dv["cmb"] = cmb.reshape(G, 1, MAXG * NBLK)

    # ---- replicated parameters (with LN of edge encoder folded) ----
    def bc(v):
        v = np.asarray(v, np.float32).reshape(1, -1)
        return np.ascontiguousarray(np.broadcast_to(v, (P, v.shape[1])))

    def row(v):
        return np.asarray(v, np.float32).reshape(1, -1)

    def b16(v):
        return np.asarray(v, np.float32).astype(np_bf16)

    e_g = np.asarray(inputs["e_g"], np.float32)       # (D,)
    e_beta = np.asarray(inputs["e_beta"], np.float32)
    We = np.asarray(inputs["We"], np.float32)         # (L, D, D)
    WeP = e_g[None, :, None] * We                     # g folded into We rows
    betaWe = np.einsum("d,ldk->lk", e_beta, We)       # (L, D) -> folded into bl

    bn_scale = (np.asarray(inputs["bn_g"], np.float32)
                / np.sqrt(np.asarray(inputs["bn_v"], np.float32) + 1e-5))
    bn_shift = (np.asarray(inputs["bn_b"], np.float32)
                + (np.asarray(inputs["cb"], np.float32)
                   - np.asarray(inputs["bn_m"], np.float32)) * bn_scale)

    bl_f = np.asarray(inputs["bl"], np.float32) + betaWe

    att = np.asarray(inputs["att"], np.float32).reshape(L, 1, D)
    att_b = np.ascontiguousarray(np.broadcast_to(att, (L, P, D)))

    # rbf rank-1 rows: log rbf[q, s] = (2g*c_q)*d_s + 1*(-g*d2_s) + (-g*c_q^2)*1
    c2g_row = (2.0 * gamma * centers).astype(np.float32).reshape(1, NUM_RBF)
    ncc_row = (-gamma * centers * centers).astype(np.float32).reshape(1, NUM_RBF)

    # eb1 column per k-tile (feature-major): [P, KD]
    eb1_col = np.asarray(inputs["eb1"], np.float32).reshape(KD, P).T.copy()
    neb1_col = np.ascontiguousarray(-eb1_col)
    eb2_col = np.asarray(inputs["eb2"], np.float32).reshape(KD, P).T.copy()

    rep = dict(
        emb_W=np.asarray(inputs["emb_W"], np.float32),
        emb_b=row(inputs["emb_b"]),
        emb_g_b=bc(inputs["emb_g"]), emb_beta_b=bc(inputs["emb_beta"]),
        eW1=b16(inputs["eW1"]),
        eW2=b16(inputs["eW2"]),
        eb1_col=eb1_col, neb1_col=neb1_col, eb2_col=eb2_col,
        c2g_row=c2g_row, ncc_row=ncc_row,
        Wl=b16(inputs["Wl"]), bl=b16(bl_f.reshape(L, 1, D)),
        Wr=b16(inputs["Wr"]), br=b16(np.asarray(inputs["br"]).reshape(L, 1, D)),
        WeP=b16(WeP),
        att_b=b16(att_b),
        bnsc_b=b16(np.broadcast_to(bn_scale.reshape(L, 1, D), (L, P, D)).copy()),
        bnsh_b=b16(np.broadcast_to(bn_shift.reshape(L, 1, D), (L, P, D)).copy()),
        pW=np.asarray(inputs["pW"], np.float32), pb=row(inputs["pb"]),
        hW1=np.asarray(inputs["hW1"], np.float32), hb1=row(inputs["hb1"]),
        hW2=np.asarray(inputs["hW2"], np.float32), hb2=row(inputs["hb2"]),
        hW3=np.pad(np.asarray(inputs["hW3"], np.float32), ((0, 64), (0, 0))).reshape(2, P).T.copy(),
        hb3=row(inputs["hb3"]),
    )

    meta = dict(n_dev=n_dev, N=N, E=E, G=G, L=L, PD=PD, N_pad=N_pad, HB=HB,
                NBLK=NBLK, CPB=CPB, EPB=EPB, x_in=x.shape[1], MAXG=MAXG)
    return meta, rep, devs


# --------------------------------------------------------------------------
# device program
# --------------------------------------------------------------------------

def build_program(meta):
    n_dev = meta["n_dev"]
    L, G = meta["L"], meta["G"]
    PD, N_pad, HB = meta["PD"], meta["N_pad"], meta["HB"]
    NBLK, CPB, EPB = meta["NBLK"], meta["CPB"], meta["EPB"]
    MAXG = meta["MAXG"]
    XIN = meta["x_in"]
    NW = (EPB + 511) // 512                            # 512-wide column groups
    wid = [(w * 512, min(512, EPB - w * 512)) for w in range(NW)]

    nc = bacc.Bacc(None, target_bir_lowering=False, debug=False)

    def inp(name, shape, dtype=f32):
        return nc.dram_tensor(name, list(shape), dtype, kind="ExternalInput")

    gidx_d = inp("gidx", (NBLK, P, EPB // 16), i16)
    rbf_rows_d = inp("rbf_rows", (NBLK, 2, EPB))
    ohg_d = inp("ohg", (NBLK, P, CPB, P), fp8)
    ohs_d = inp("ohs", (NBLK, P, CPB, P), fp8)
    xT_d = inp("xT", (XIN, PD))
    oh_d = inp("oh", (NBLK, P, G))
    maskAB_d = inp("maskAB", (NBLK, P, MAXG))
    cmb_d = inp("cmb", (G, 1, MAXG * NBLK))

    emb_W_d = inp("emb_W", (XIN, D))
    emb_b_d = inp("emb_b", (1, D))
    emb_g_b_d = inp("emb_g_b", (P, D))
    emb_beta_b_d = inp("emb_beta_b", (P, D))
    eW1_d = inp("eW1", (NUM_RBF, D), bf16)
    eW2_d = inp("eW2", (D, D), bf16)
    eb1_col_d = inp("eb1_col", (P, KD))
    neb1_col_d = inp("neb1_col", (P, KD))
    eb2_col_d = inp("eb2_col", (P, KD))
    c2g_row_d = inp("c2g_row", (1, NUM_RBF))
    ncc_row_d = inp("ncc_row", (1, NUM_RBF))
    Wl_d = inp("Wl", (L, D, D), bf16)
    bl_d = inp("bl", (L, 1, D), bf16)
    Wr_d = inp("Wr", (L, D, D), bf16)
    br_d = inp("br", (L, 1, D), bf16)
    WeP_d = inp("WeP", (L, D, D), bf16)
    att_b_d = inp("att_b", (L, P, D), bf16)
    bnsc_b_d = inp("bnsc_b", (L, P, D), bf16)
    bnsh_b_d = inp("bnsh_b", (L, P, D), bf16)
    pW_d = inp("pW", (2 * D, D))
    pb_d = inp("pb", (1, D))
    hW1_d = inp("hW1", (D, D))
    hb1_d = inp("hb1", (1, D))
    hW2_d = inp("hW2", (D, D // 2))
    hb2_d = inp("hb2", (1, D // 2))
    hW3_d = inp("hW3", (P, 2))
    hb3_d = inp("hb3", (1, 1))

    out_d = nc.dram_tensor("out", [G], f32, kind="ExternalOutput")

    encT_d = nc.dram_tensor("encT", [NBLK, KD, P, EPB], bf16)
    xl_shard_d = nc.dram_tensor("xl_shard", [PD, D], bf16)
    shared_as = "Shared" if n_dev > 4 else "Local"
    xl_full_d = nc.dram_tensor("xl_full", [N_pad, D], bf16, addr_space=shared_as)
    pool_part_d = nc.dram_tensor("pool_part", [2 * D + 1, G], f32)
    pool_all_d = nc.dram_tensor("pool_all", [n_dev * (2 * D + 1), G], f32, addr_space=shared_as)

    rg = [list(range(n_dev))]

    with tile.TileContext(nc) as tc:
        with (
            tc.tile_pool(name="consts", bufs=1) as consts,
            tc.tile_pool(name="hpool", bufs=1) as hpool,
        ):
            nc.gpsimd.load_library(mlp_lib)
            ident = consts.tile([P, P], f32, tag="ident")
            make_identity(nc, ident)
            ident_b = consts.tile([P, P], bf16, tag="ident_b")
            make_identity(nc, ident_b)
            ones_row = consts.tile([1, P], f32, tag="ones_row")
            nc.vector.memset(ones_row[:], 1.0)
            ones_col = consts.tile([P, 1], f32, tag="ones_col")
            nc.vector.memset(ones_col[:], 1.0)
            ones_row_b = consts.tile([1, P], bf16, tag="ones_row_b")
            nc.vector.memset(ones_row_b[:], 1.0)
            invD_col_b = consts.tile([P, 1], bf16, tag="invD_col_b")
            nc.vector.memset(invD_col_b[:], 1.0 / D)
            ones_col_b = consts.tile([P, 1], bf16, tag="ones_col_b")
            nc.vector.memset(ones_col_b[:], 1.0)
            onesw_row = consts.tile([1, 512], f32, tag="onesw_row")
            nc.vector.memset(onesw_row[:], 1.0)
            eps_col = consts.tile([P, 1], f32, tag="eps_col")
            nc.vector.memset(eps_col[:], 1e-5)

            h_sb = [hpool.tile([P, D], f32, tag=f"h{b}", name=f"h{b}")
                    for b in range(NBLK)]

            silu_n = [0]

            def emit_silu(pool, out_ap, in_ap, shape):
                # silu(x) = x / (1 + exp(-x)); exp on ACT, rest split DVE/Pool
                silu_n[0] += 1
                sn = silu_n[0]
                ex = pool.tile(shape, f32, tag="silu_ex", name=f"silu_ex{sn}")
                nc.scalar.activation(ex[:], in_ap, AF.Exp, scale=-1.0)
                nc.vector.tensor_scalar(out=ex[:], in0=ex[:], scalar1=1.0,
                                        scalar2=None, op0=OP.add)
                rcp = pool.tile(shape, f32, tag="silu_rc", name=f"silu_rc{sn}")
                nc.vector.reciprocal_approx_fast(rcp[:], ex[:])
                nc.vector.tensor_tensor(out=out_ap, in0=in_ap, in1=rcp[:], op=OP.mult)

            # =========================================================
            # Stage B: node embedding  h0 = silu(LN(x @ emb_W + emb_b))
            # =========================================================
            with (
                tc.tile_pool(name="embsb", bufs=2) as embsb,
                tc.tile_pool(name="embc", bufs=1) as embc,
                tc.tile_pool(name="embps", bufs=2, space="PSUM") as embps,
            ):
                xT_sb = embc.tile([XIN, PD], f32, tag="xT")
                nc.sync.dma_start(xT_sb[:], xT_d[:, :])
                embW_sb = embc.tile([XIN, D], f32, tag="embW")
                nc.sync.dma_start(embW_sb[:], emb_W_d[:, :])
                embb_sb = embc.tile([1, D], f32, tag="embb")
                nc.sync.dma_start(embb_sb[:], emb_b_d[:, :])
                emb_g_sb = embc.tile([P, D], f32, tag="embg")
                nc.sync.dma_start(emb_g_sb[:], emb_g_b_d[:, :])
                emb_beta_sb = embc.tile([P, D], f32, tag="embbeta")
                nc.sync.dma_start(emb_beta_sb[:], emb_beta_b_d[:, :])

                for b in range(NBLK):
                    ps = embps.tile([P, D], f32, tag="ps")
                    nc.tensor.matmul(ps[:], xT_sb[:, b * P:(b + 1) * P], embW_sb[:],
                                     start=True, stop=False)
                    nc.tensor.matmul(ps[:], ones_row[:, :P], embb_sb[:],
                                     start=False, stop=True)
                    mu = embsb.tile([P, 1], f32, tag="mu")
                    nc.vector.tensor_reduce(out=mu[:], in_=ps[:],
                                            axis=mybir.AxisListType.X, op=OP.add)
                    nc.vector.tensor_scalar(out=mu[:], in0=mu[:], scalar1=1.0 / D,
                                            scalar2=None, op0=OP.mult)
                    xc = embsb.tile([P, D], f32, tag="xc")
                    nc.vector.tensor_scalar(out=xc[:], in0=ps[:], scalar1=mu[:, :1],
                                            scalar2=None, op0=OP.subtract)
                    sq = embsb.tile([P, D], f32, tag="sq")
                    var = embsb.tile([P, 1], f32, tag="var")
                    nc.scalar.activation(sq[:], xc[:], AF.Square, accum_out=var[:, :1])
                    lnv = embsb.tile([P, 1], f32, tag="lnv")
                    nc.scalar.activation(lnv[:], var[:], AF.Ln, scale=1.0 / D, bias=eps_col[:, :1])
                    rstd = embsb.tile([P, 1], f32, tag="rstd")
                    nc.scalar.activation(rstd[:], lnv[:], AF.Exp, scale=-0.5)
                    nc.vector.tensor_scalar(out=xc[:], in0=xc[:], scalar1=rstd[:, :1],
                                            scalar2=None, op0=OP.mult)
                    nc.vector.tensor_tensor(out=xc[:], in0=xc[:], in1=emb_g_sb[:], op=OP.mult)
                    nc.vector.tensor_tensor(out=xc[:], in0=xc[:], in1=emb_beta_sb[:], op=OP.add)
                    emit_silu(embsb, h_sb[b][:], xc[:], [P, D])

            # =========================================================
            # Stage C: edge encoder -> encT  (feature-major, LN folded)
            # =========================================================
            with (
                tc.tile_pool(name="encw", bufs=1) as encw,
                tc.tile_pool(name="encbig", bufs=1) as encbig,
                tc.tile_pool(name="encsm", bufs=2) as encsm,
                tc.tile_pool(name="encps", bufs=1, space="PSUM") as encps,
            ):
                eW1_sb = encw.tile([NUM_RBF, D], bf16, tag="eW1")
                nc.sync.dma_start(eW1_sb[:], eW1_d[:, :])
                eW2_sb = [encw.tile([P, D], bf16, tag=f"eW2_{k}", name=f"eW2_{k}")
                          for k in range(KD)]
                for k in range(KD):
                    nc.sync.dma_start(eW2_sb[k][:], eW2_d[k * P:(k + 1) * P, :])
                eb1_sb = encw.tile([P, KD], f32, tag="eb1c")
                nc.sync.dma_start(eb1_sb[:], eb1_col_d[:, :])
                neb1_sb = encw.tile([P, KD], f32, tag="neb1c")
                nc.sync.dma_start(neb1_sb[:], neb1_col_d[:, :])
                eb2_sb = encw.tile([P, KD], f32, tag="eb2c")
                nc.sync.dma_start(eb2_sb[:], eb2_col_d[:, :])
                c2g_sb = encw.tile([1, NUM_RBF], f32, tag="c2g")
                nc.sync.dma_start(c2g_sb[:], c2g_row_d[:, :])
                ncc_sb = encw.tile([1, NUM_RBF], f32, tag="ncc")
                nc.sync.dma_start(ncc_sb[:], ncc_row_d[:, :])
                ones100 = encw.tile([1, NUM_RBF], f32, tag="ones100")
                nc.vector.memset(ones100[:], 1.0)

                for b in range(NBLK):
                    rowd = encsm.tile([1, EPB], f32, tag="rowd")
                    nc.sync.dma_start(rowd[:], rbf_rows_d[b, 0:1, :])
                    rownd = encsm.tile([1, EPB], f32, tag="rownd")
                    nc.sync.dma_start(rownd[:], rbf_rows_d[b, 1:2, :])
                    rbf_sb = encbig.tile([NUM_RBF, EPB], bf16, tag="rbf", bufs=2)
                    # --- rbf = exp(rank-1 logits) ---
                    for (w0, wn) in wid:
                        pr = encps.tile([P, 512], f32, tag="pe", bufs=4)
                        nc.tensor.matmul(pr[:NUM_RBF, :wn], c2g_sb[:],
                                         rowd[:, w0:w0 + wn], start=True, stop=False)
                        nc.tensor.matmul(pr[:NUM_RBF, :wn], ones100[:],
                                         rownd[:, w0:w0 + wn], start=False, stop=False)
                        nc.tensor.matmul(pr[:NUM_RBF, :wn], ncc_sb[:],
                                         onesw_row[:, :wn], start=False, stop=True)
                        nc.scalar.activation(rbf_sb[:, w0:w0 + wn], pr[:NUM_RBF, :wn],
                                             AF.Exp)
                    # --- e1 = silu(eW1.T @ rbf + eb1)  (feature-major) ---
                    e1_sb = [encbig.tile([P, EPB], bf16, tag=f"e1_{k}", name=f"e1_{k}",
                                         bufs=2) for k in range(KD)]
                    for k in range(KD):
                        for (w0, wn) in wid:
                            pe = encps.tile([P, 512], f32, tag="pe", bufs=4)
                            nc.tensor.matmul(pe[:, :wn], eW1_sb[:, k * P:(k + 1) * P],
                                             rbf_sb[:, w0:w0 + wn], start=True, stop=True)
                            ex = encsm.tile([P, 512], f32, tag="ex")
                            nc.scalar.activation(ex[:, :wn], pe[:, :wn], AF.Exp,
                                                 scale=-1.0, bias=neb1_sb[:, k:k + 1])
                            nc.vector.tensor_scalar(out=ex[:, :wn], in0=ex[:, :wn],
                                                    scalar1=1.0, scalar2=None, op0=OP.add)
                            rc = encsm.tile([P, 512], f32, tag="rc")
                            nc.vector.reciprocal_approx_fast(rc[:, :wn], ex[:, :wn])
                            nc.vector.scalar_tensor_tensor(
                                out=e1_sb[k][:, w0:w0 + wn], in0=pe[:, :wn],
                                scalar=eb1_sb[:, k:k + 1], in1=rc[:, :wn],
                                op0=OP.add, op1=OP.mult)
                    # --- e2 = e1 @ eW2 + eb2 (feature-major), then LN fold ---
                    xc_sb = [encbig.tile([P, EPB], bf16, tag=f"xc_{m}", name=f"xc_{m}",
                                         bufs=2) for m in range(KD)]
                    mu_row = encsm.tile([1, EPB], f32, tag="mu_row")
                    for m in range(KD):
                        for (w0, wn) in wid:
                            pe = encps.tile([P, 512], f32, tag="pe", bufs=4)
                            for k in range(KD):
                                nc.tensor.matmul(pe[:, :wn],
                                                 eW2_sb[k][:, m * P:(m + 1) * P],
                                                 e1_sb[k][:, w0:w0 + wn],
                                                 start=(k == 0), stop=(k == KD - 1))
                            # e2 with bias -> bf16 (raw, for stats)
                            nc.vector.tensor_scalar(out=xc_sb[m][:, w0:w0 + wn],
                                                    in0=pe[:, :wn],
                                                    scalar1=eb2_sb[:, m:m + 1],
                                                    scalar2=None, op0=OP.add)
                    # --- stats: mu ---
                    for (w0, wn) in wid:
                        pmu = encps.tile([1, 512], f32, tag="pmu", bufs=2)
                        for m in range(KD):
                            nc.tensor.matmul(pmu[:, :wn], invD_col_b[:],
                                             xc_sb[m][:, w0:w0 + wn],
                                             start=(m == 0), stop=(m == KD - 1))
                        nc.vector.tensor_copy(mu_row[:, w0:w0 + wn], pmu[:, :wn])
                    # center: xc = e2 - bc(mu)
                    for (w0, wn) in wid:
                        pbc = encps.tile([P, 512], f32, tag="pe", bufs=4)
                        nc.tensor.matmul(pbc[:, :wn], ones_row[:],
                                         mu_row[:, w0:w0 + wn], start=True, stop=True)
                        for m in range(KD):
                            nc.vector.tensor_tensor(out=xc_sb[m][:, w0:w0 + wn],
                                                    in0=xc_sb[m][:, w0:w0 + wn],
                                                    in1=pbc[:, :wn], op=OP.subtract)
                    # var & rstd rows
                    rstd_row = encsm.tile([1, EPB], f32, tag="rstd_row")
                    for (w0, wn) in wid:
                        pvar = encps.tile([1, 512], f32, tag="pmu", bufs=2)
                        for m in range(KD):
                            sq = encsm.tile([P, 512], bf16, tag="sq")
                            nc.vector.tensor_tensor(out=sq[:, :wn],
                                                    in0=xc_sb[m][:, w0:w0 + wn],
                                                    in1=xc_sb[m][:, w0:w0 + wn],
                                                    op=OP.mult)
                            nc.tensor.matmul(pvar[:, :wn], ones_col_b[:],
                                             sq[:, :wn],
                                             start=(m == 0), stop=(m == KD - 1))
                        nc.scalar.activation(rstd_row[:, w0:w0 + wn], pvar[:, :wn],
                                             AF.Ln, scale=1.0 / D, bias=eps_col[:1, :1])
                    nc.scalar.activation(rstd_row[:], rstd_row[:], AF.Exp, scale=-0.5)
                    # apply rstd and store
                    for (w0, wn) in wid:
                        pbc = encps.tile([P, 512], f32, tag="pe", bufs=4)
                        nc.tensor.matmul(pbc[:, :wn], ones_row[:],
                                         rstd_row[:, w0:w0 + wn], start=True, stop=True)
                        for m in range(KD):
                            nc.vector.tensor_tensor(out=xc_sb[m][:, w0:w0 + wn],
                                                    in0=xc_sb[m][:, w0:w0 + wn],
                                                    in1=pbc[:, :wn], op=OP.mult)
                    for m in range(KD):
                        nc.sync.dma_start(encT_d[b, m], xc_sb[m][:])

            # =========================================================
            # Main layers
            # =========================================================
            with (
                tc.tile_pool(name="xrpool", bufs=1) as xrpool,
                tc.tile_pool(name="lw", bufs=2) as lw,
                tc.tile_pool(name="lsb", bufs=2) as lsb,
                tc.tile_pool(name="gat", bufs=2) as gat,
                tc.tile_pool(name="eetp", bufs=2) as eetp,
                tc.tile_pool(name="lps", bufs=3, space="PSUM") as lps,
                tc.tile_pool(name="lpt", bufs=2, space="PSUM") as lpt,
                tc.tile_pool(name="lpo", bufs=2, space="PSUM") as lpo,
            ):
                xr_sb = [xrpool.tile([P, D], bf16, tag=f"xr{b}", name=f"xr{b}")
                         for b in range(NBLK)]
                for layer in range(L):
                    Wl_sb = [lw.tile([P, D], bf16, tag=f"Wl{k}", name=f"Wl{k}")
                             for k in range(KD)]
                    Wr_sb = [lw.tile([P, D], bf16, tag=f"Wr{k}", name=f"Wr{k}")
                             for k in range(KD)]
                    We_sb = [lw.tile([P, D], bf16, tag=f"We{k}", name=f"We{k}")
                             for k in range(KD)]
                    for k in range(KD):
                        nc.sync.dma_start(Wl_sb[k][:], Wl_d[layer, k * P:(k + 1) * P, :])
                        nc.sync.dma_start(Wr_sb[k][:], Wr_d[layer, k * P:(k + 1) * P, :])
                        nc.sync.dma_start(We_sb[k][:], WeP_d[layer, k * P:(k + 1) * P, :])
                    bl_sb = lw.tile([1, D], bf16, tag="bl")
                    nc.sync.dma_start(bl_sb[:], bl_d[layer])
                    br_sb = lw.tile([1, D], bf16, tag="br")
                    nc.sync.dma_start(br_sb[:], br_d[layer])
                    attb_sb = lw.tile([P, D], bf16, tag="attb")
                    nc.sync.dma_start(attb_sb[:], att_b_d[layer])
                    bnsc_sb = lw.tile([P, D], bf16, tag="bnsc")
                    nc.sync.dma_start(bnsc_sb[:], bnsc_b_d[layer])
                    bnsh_sb = lw.tile([P, D], bf16, tag="bnsh")
                    nc.sync.dma_start(bnsh_sb[:], bnsh_b_d[layer])

                    # ---- stage D: xl (+xr), AllGather in two halves ----
                    for b in range(NBLK):
                        hT = []
                        for k in range(KD):
                            pt = lpt.tile([P, P], f32, tag="pt")
                            nc.tensor.transpose(pt[:], h_sb[b][:, k * P:(k + 1) * P], ident[:])
                            t = lsb.tile([P, P], bf16, tag=f"hT{k}", name=f"hT{k}")
                            nc.vector.tensor_copy(t[:], pt[:])
                            hT.append(t)
                        pxl = lps.tile([P, D], f32, tag="ps")
                        for k in range(KD):
                            nc.tensor.matmul(pxl[:], hT[k][:], Wl_sb[k][:],
                                             start=(k == 0), stop=False)
                        nc.tensor.matmul(pxl[:], ones_row_b[:, :P], bl_sb[:],
                                         start=False, stop=True)
                        xl_t = lsb.tile([P, D], bf16, tag="xl_t")
                        nc.scalar.copy(xl_t[:], pxl[:])
                        nc.sync.dma_start(xl_shard_d[b * P:(b + 1) * P, :], xl_t[:])
                        pxr = lps.tile([P, D], f32, tag="ps")
                        for k in range(KD):
                            nc.tensor.matmul(pxr[:], hT[k][:], Wr_sb[k][:],
                                             start=(k == 0), stop=False)
                        nc.tensor.matmul(pxr[:], ones_row_b[:, :P], br_sb[:],
                                         start=False, stop=True)
                        nc.vector.tensor_copy(xr_sb[b][:], pxr[:])
                        if b == NBLK // 2 - 1:
                            nc.gpsimd.collective_compute(
                                "AllGather", OP.bypass, replica_groups=rg,
                                ins=[xl_shard_d[0:HB, :]],
                                outs=[xl_full_d[0:n_dev * HB, :]],
                            )
                    nc.gpsimd.collective_compute(
                        "AllGather", OP.bypass, replica_groups=rg,
                        ins=[xl_shard_d[HB:PD, :]],
                        outs=[xl_full_d[n_dev * HB:2 * n_dev * HB, :]],
                    )

                    # ---- stage E: edge message passing ----
                    for b in range(NBLK):
                        gix = gat.tile([P, EPB // 16], i16, tag="gix")
                        nc.sync.dma_start(gix[:], gidx_d[b])
                        ohg = gat.tile([P, CPB, P], fp8, tag="ohg")
                        nc.sync.dma_start(ohg[:], ohg_d[b])
                        ohs = gat.tile([P, CPB, P], fp8, tag="ohs")
                        nc.sync.dma_start(ohs[:], ohs_d[b])
                        eet = [eetp.tile([P, EPB], bf16, tag=f"eet{k}", name=f"eet{k}")
                               for k in range(KD)]
                        for k in range(KD):
                            nc.sync.dma_start(eet[k][:], encT_d[b, k])
                        xsg = eetp.tile([P, CPB, D], bf16, tag="xsg")
                        nc.gpsimd.dma_gather(xsg[:], xl_full_d[:, :], gix[:], EPB, EPB, D,
                                             single_packet=False)
                        m_all = eetp.tile([P, CPB, D], bf16, tag="m_all")
                        z_all = eetp.tile([P, CPB, D + 16], bf16, tag="z_all")
                        lg = lsb.tile([P, CPB * H], f32, tag="lg")
                        psum_o = lpo.tile([P, D + H], f32, tag="po")
                        for c in range(CPB):
                            psum_s = lps.tile([P, D], f32, tag="ps")
                            for k in range(KD):
                                nc.tensor.matmul(psum_s[:], eet[k][:, c * P:(c + 1) * P],
                                                 We_sb[k][:], start=(k == 0), stop=False)
                            nc.tensor.matmul(psum_s[:], ohg[:, c, :], xr_sb[b][:],
                                             start=False, stop=False)
                            nc.tensor.matmul(psum_s[:], ident_b[:], xsg[:, c, :],
                                             start=False, stop=True)
                            nc.scalar.activation(m_all[:, c, :], psum_s[:], AF.Prelu,
                                                 alpha=0.2)
                        # block-wide logits/softmax/messages
                        nc.vector.tensor_tensor(
                            out=m_all[:],
                            in0=m_all[:],
                            in1=attb_sb[:].rearrange("p (o d) -> p o d", o=1)
                                .to_broadcast([P, CPB, D]),
                            op=OP.mult)
                        nc.vector.tensor_reduce(
                            out=lg[:],
                            in_=m_all[:].rearrange("p c (h cc) -> p (c h) cc", cc=C),
                            axis=mybir.AxisListType.X, op=OP.add)
                        el = z_all[:, :, D:D + H]
                        nc.scalar.activation(el, lg[:].rearrange("p (c h) -> p c h", h=H),
                                             AF.Exp)
                        nc.vector.tensor_tensor(
                            out=z_all[:, :, :D].rearrange("p c (h cc) -> p c h cc", cc=C),
                            in0=xsg[:].rearrange("p c (h cc) -> p c h cc", cc=C),
                            in1=el.rearrange("p c (h o) -> p c h o", o=1)
                                .to_broadcast([P, CPB, H, C]),
                            op=OP.mult)
                        for c in range(CPB):
                            nc.tensor.matmul(psum_o[:], ohs[:, c, :], z_all[:, c, :D + H],
                                             start=(c == 0), stop=(c == CPB - 1))
                        # ---- block epilogue ----
                        den = lsb.tile([P, H], f32, tag="den")
                        nc.vector.tensor_scalar(out=den[:], in0=psum_o[:, D:],
                                                scalar1=1e-16, scalar2=None, op0=OP.add)
                        rec = lsb.tile([P, H], f32, tag="rec")
                        nc.vector.reciprocal_approx_fast(rec[:], den[:])
                        o1 = lsb.tile([P, D], f32, tag="o1")
                        rec_b = rec[:].rearrange("p (h o) -> p h o", o=1).to_broadcast([P, H, C])
                        nc.vector.tensor_tensor(
                            out=o1[:].rearrange("p (h cc) -> p h cc", h=H),
                            in0=psum_o[:, :D].rearrange("p (h cc) -> p h cc", h=H),
                            in1=rec_b, op=OP.mult)
                        nc.vector.tensor_tensor(out=o1[:], in0=o1[:], in1=bnsc_sb[:], op=OP.mult)
                        nc.vector.tensor_tensor(out=o1[:], in0=o1[:], in1=bnsh_sb[:], op=OP.add)
                        # silu + residual
                        ex = lsb.tile([P, D], f32, tag="silu_ex")
                        nc.scalar.activation(ex[:], o1[:], AF.Exp, scale=-1.0)
                        nc.vector.tensor_scalar(out=ex[:], in0=ex[:], scalar1=1.0,
                                                scalar2=None, op0=OP.add)
                        rcp = lsb.tile([P, D], f32, tag="silu_rc")
                        nc.vector.reciprocal_approx_fast(rcp[:], ex[:])
                        nc.vector.tensor_tensor(out=o1[:], in0=o1[:], in1=rcp[:], op=OP.mult)
                        nc.vector.tensor_tensor(out=h_sb[b][:], in0=h_sb[b][:], in1=o1[:],
                                                op=OP.add)

            # =========================================================
            # Stage F: pooling + head
            # =========================================================
            with (
                tc.tile_pool(name="fsb", bufs=3) as fsb,
                tc.tile_pool(name="fkeep", bufs=1) as fkeep,
                tc.tile_pool(name="fps", bufs=2, space="PSUM") as fps,
                tc.tile_pool(name="fsum", bufs=1, space="PSUM") as fsum,
            ):
                psum_sum = fsum.tile([G, D], f32, tag="psum_sum")
                psum_cnt = fsum.tile([1, G], f32, tag="psum_cnt")
                bm = [fkeep.tile([P, MAXG * NBLK], f32, tag=f"bm{k}", name=f"bm{k}")
                      for k in range(KD)]
                for b in range(NBLK):
                    ohb = fsb.tile([P, G], f32, tag="ohb")
                    nc.sync.dma_start(ohb[:], oh_d[b])
                    mab = fsb.tile([P, MAXG], f32, tag="mab")
                    nc.sync.dma_start(mab[:], maskAB_d[b])
                    nc.tensor.matmul(psum_sum[:], ohb[:], h_sb[b][:],
                                     start=(b == 0), stop=(b == NBLK - 1))
                    nc.tensor.matmul(psum_cnt[:], ones_col[:, :1], ohb[:],
                                     start=(b == 0), stop=(b == NBLK - 1))
                    for half in range(MAXG):
                        mh = fsb.tile([P, D], f32, tag="mh")
                        nc.vector.tensor_scalar(out=mh[:], in0=h_sb[b][:],
                                                scalar1=mab[:, half:half + 1], scalar2=None,
                                                op0=OP.add)
                        for k in range(KD):
                            pt = fps.tile([P, P], f32, tag="pt")
                            nc.tensor.transpose(pt[:], mh[:, k * P:(k + 1) * P], ident[:])
                            mt = fsb.tile([P, P], f32, tag="mt")
                            nc.vector.tensor_copy(mt[:], pt[:])
                            nc.vector.tensor_reduce(
                                out=bm[k][:, MAXG * b + half:MAXG * b + half + 1],
                                in_=mt[:], axis=mybir.AxisListType.X, op=OP.max)
                gmaxT = [fkeep.tile([P, G], f32, tag=f"gmaxT{k}", name=f"gmaxT{k}")
                         for k in range(KD)]
                for g in range(G):
                    cr = fsb.tile([1, MAXG * NBLK], f32, tag="cr")
                    nc.sync.dma_start(cr[:], cmb_d[g])
                    pc = fps.tile([P, MAXG * NBLK], f32, tag="pt")
                    nc.tensor.matmul(pc[:], ones_row[:, :P], cr[:], start=True, stop=True)
                    for k in range(KD):
                        mm = fsb.tile([P, MAXG * NBLK], f32, tag="mm")
                        nc.vector.tensor_tensor(out=mm[:], in0=bm[k][:], in1=pc[:], op=OP.add)
                        nc.vector.tensor_reduce(out=gmaxT[k][:, g:g + 1], in_=mm[:],
                                                axis=mybir.AxisListType.X, op=OP.max)
                sum_sb = fsb.tile([G, D], f32, tag="sum_sb")
                nc.vector.tensor_copy(sum_sb[:], psum_sum[:])
                for k in range(KD):
                    pt = fps.tile([P, G], f32, tag="pt")
                    nc.tensor.transpose(pt[:, :G], sum_sb[:, k * P:(k + 1) * P], ident[:G, :G])
                    st = fsb.tile([P, G], f32, tag="st")
                    nc.vector.tensor_copy(st[:], pt[:, :G])
                    nc.sync.dma_start(pool_part_d[k * P:(k + 1) * P, :], st[:])
                    nc.sync.dma_start(pool_part_d[D + k * P:D + (k + 1) * P, :], gmaxT[k][:])
                cntT = fsb.tile([1, G], f32, tag="cntT")
                nc.vector.tensor_copy(cntT[:], psum_cnt[:])
                nc.sync.dma_start(pool_part_d[2 * D:2 * D + 1, :], cntT[:])

                nc.gpsimd.collective_compute(
                    "AllGather", OP.bypass, replica_groups=rg,
                    ins=[pool_part_d[:, :]], outs=[pool_all_d[:, :]],
                )

                n_dev_ = n_dev
                STRIDE = 2 * D + 1
                meanT = [fkeep.tile([P, G], f32, tag=f"meanT{k}", name=f"meanT{k}")
                         for k in range(KD)]
                maxT = [fkeep.tile([P, G], f32, tag=f"maxT{k}", name=f"maxT{k}")
                        for k in range(KD)]
                cnt_tot = fkeep.tile([1, G], f32, tag="cnt_tot")
                for dv in range(n_dev_):
                    base = dv * STRIDE
                    for k in range(KD):
                        ts = fsb.tile([P, G], f32, tag="ts")
                        nc.sync.dma_start(ts[:], pool_all_d[base + k * P:base + (k + 1) * P, :])
                        tm = fsb.tile([P, G], f32, tag="tm")
                        nc.sync.dma_start(tm[:], pool_all_d[base + D + k * P:base + D + (k + 1) * P, :])
                        if dv == 0:
                            nc.vector.tensor_copy(meanT[k][:], ts[:])
                            nc.vector.tensor_copy(maxT[k][:], tm[:])
                        else:
                            nc.vector.tensor_tensor(out=meanT[k][:], in0=meanT[k][:],
                                                    in1=ts[:], op=OP.add)
                            nc.vector.tensor_tensor(out=maxT[k][:], in0=maxT[k][:],
                                                    in1=tm[:], op=OP.max)
                    tc_ = fsb.tile([1, G], f32, tag="tc_")
                    nc.sync.dma_start(tc_[:], pool_all_d[base + 2 * D:base + 2 * D + 1, :])
                    if dv == 0:
                        nc.vector.tensor_copy(cnt_tot[:], tc_[:])
                    else:
                        nc.vector.tensor_tensor(out=cnt_tot[:], in0=cnt_tot[:], in1=tc_[:],
                                                op=OP.add)
                nc.vector.tensor_scalar(out=cnt_tot[:], in0=cnt_tot[:], scalar1=1.0,
                                        scalar2=None, op0=OP.max)
                inv_cnt = fkeep.tile([1, G], f32, tag="inv_cnt")
                nc.vector.reciprocal(inv_cnt[:], cnt_tot[:])
                pic = fps.tile([P, G], f32, tag="pt")
                nc.tensor.matmul(pic[:], ones_row[:, :P], inv_cnt[:], start=True, stop=True)
                for k in range(KD):
                    nc.vector.tensor_tensor(out=meanT[k][:], in0=meanT[k][:], in1=pic[:],
                                            op=OP.mult)
                hgT = meanT + maxT

                pW_sb = [fkeep.tile([P, D], f32, tag=f"pW{k}", name=f"pW{k}")
                         for k in range(2 * KD)]
                for k in range(2 * KD):
                    nc.sync.dma_start(pW_sb[k][:], pW_d[k * P:(k + 1) * P, :])
                pb_sb = fkeep.tile([1, D], f32, tag="pb")
                nc.sync.dma_start(pb_sb[:], pb_d[:, :])
                hW1_sb = [fkeep.tile([P, D], f32, tag=f"hW1_{k}", name=f"hW1_{k}")
                          for k in range(KD)]
                for k in range(KD):
                    nc.sync.dma_start(hW1_sb[k][:], hW1_d[k * P:(k + 1) * P, :])
                hb1_sb = fkeep.tile([1, D], f32, tag="hb1")
                nc.sync.dma_start(hb1_sb[:], hb1_d[:, :])
                hW2_sb = [fkeep.tile([P, D // 2], f32, tag=f"hW2_{k}", name=f"hW2_{k}")
                          for k in range(KD)]
                for k in range(KD):
                    nc.sync.dma_start(hW2_sb[k][:], hW2_d[k * P:(k + 1) * P, :])
                hb2_sb = fkeep.tile([1, D // 2], f32, tag="hb2")
                nc.sync.dma_start(hb2_sb[:], hb2_d[:, :])
                hW3_sb = fkeep.tile([P, 2], f32, tag="hW3")
                nc.sync.dma_start(hW3_sb[:], hW3_d[:, :].rearrange("(k p) o -> p (k o)", p=P))
                hb3_sb = fkeep.tile([1, 1], f32, tag="hb3")
                nc.sync.dma_start(hb3_sb[:], hb3_d[:, :])

                def mlp_layer(in_tiles, W_tiles, b_row, out_feats, lid, act=True):
                    outs = []
                    n_out_tiles = (out_feats + P - 1) // P
                    for m in range(n_out_tiles):
                        mw = min(P, out_feats - m * P)
                        pm = fps.tile([P, G], f32, tag="ph", name=f"ph{lid}_{m}")
                        for k, (it, wt) in enumerate(zip(in_tiles, W_tiles)):
                            nc.tensor.matmul(pm[:mw, :], wt[:, m * P:m * P + mw], it[:],
                                             start=(k == 0), stop=False)
                        nc.tensor.matmul(pm[:mw, :], b_row[:, m * P:m * P + mw],
                                         ones_row[:, :G], start=False, stop=True)
                        ot = fkeep.tile([P, G], f32, tag=f"ot{lid}_{m}", name=f"ot{lid}_{m}")
                        if mw < P:
                            nc.vector.memset(ot[mw:, :], 0.0)
                        if act:
                            emit_silu(fsb, ot[:mw, :], pm[:mw, :], [mw, G])
                        else:
                            nc.vector.tensor_copy(ot[:mw, :], pm[:mw, :])
                        outs.append(ot)
                    return outs

                h1 = mlp_layer(hgT, pW_sb, pb_sb, D, 1)
                h2 = mlp_layer(h1, hW1_sb, hb1_sb, D, 2)
                h3 = mlp_layer(h2, hW2_sb, hb2_sb, D // 2, 3)
                pf = fps.tile([1, G], f32, tag="pf")
                nc.tensor.matmul(pf[:], hW3_sb[:, 0:1], h3[0][:], start=True, stop=False)
                nc.tensor.matmul(pf[:], hW3_sb[:, 1:2], h3[1][:], start=False, stop=False)
                nc.tensor.matmul(pf[:], hb3_sb[:, :1], ones_row[:, :G], start=False, stop=True)
                fo = fsb.tile([1, G], f32, tag="fo")
                nc.vector.tensor_copy(fo[:], pf[:])
                nc.sync.dma_start(out_d[:].rearrange("(o g) -> o g", o=1), fo[:])

    nc.compile()
    return nc


# --------------------------------------------------------------------------
# entry point
# --------------------------------------------------------------------------

def kernel(**inputs):
    n_dev = 8
    meta, rep, devs = prep_host(inputs, n_dev)
    nc = build_program(meta)

    in_maps = []
    for d in range(n_dev):
        m = dict(rep)
        m.update(devs[d])
        in_maps.append(m)

    global LAST_RESULTS
    res = run_bass_kernel_spmd(nc, in_maps, core_ids=list(range(n_dev)),
                               trace=TRACE)
    LAST_RESULTS = res
    out = np.asarray(res.results[0]["out"], np.float32)
    return out
